# revision 2
# baseline (speedup 1.0000x reference)
import sys

if "/opt/trn_rl_repo" not in sys.path:
    sys.path.insert(0, "/opt/trn_rl_repo")

import os
import numpy as np
import ml_dtypes

NO_CC = os.environ.get("KNO_CC", "0") == "1"
NO_DS = os.environ.get("KNO_DS", "0") == "1"

BF16 = ml_dtypes.bfloat16

# Problem constants (nn_BiLSTM_77034533421798)
T_FULL = 512
B_FULL = 128
H = 400
G = 1600  # 4*H
BL = 32   # batch per core (4 quarters x 2 directions on 8 cores)
NEG = 1.0e9

# hidden-unit halves; half0 gate order [i, f, o, g], half1 [i, f, g, o]
# (half1's sigma_o tail overflows into the pssa psum bank, off the c-chain)
HALVES = [(0, 256), (256, 400)]
PT_H = {0: ([0, 1, 3, 2], [1, 1, 0, 1][3:] and [True, True, True, False]),
        1: ([0, 1, 2, 3], [True, True, False, True])}


def _perm_and_sigma():
    perm = np.empty(G, np.int64)
    sigma = np.zeros(G, bool)
    n = 0
    for hi, (u0, u1) in enumerate(HALVES):
        pts, sigs = PT_H[hi]
        for tau in range(4):
            for u in range(u0, u1):
                perm[n] = pts[tau] * H + u
                sigma[n] = sigs[tau]
                n += 1
    return perm, sigma


PERM, SIGMA = _perm_and_sigma()
# stream col layout: half0 [0:1024) = i0 f0 o0 g0 (256 each),
#                    half1 [1024:1600) = i1 f1 o1 g1 (144 each)
# psum: acc1 pair holds cols 0:1536 (3 banks each); the last 64 cols (g1
# tail) plus the transpose scratch all pack into one extra bank (pss).
NB1 = [(0, 512), (512, 512), (1024, 512)]  # acc1 col blocks (bank-aligned)
HCH = [(0, 128), (128, 256), (256, 384), (384, 400)]  # h contraction chunks
A_ROWS = [(0, 128), (128, 256), (256, 384), (384, 402)]
W1_ROWS = [(0, 128), (128, 256), (256, 384), (384, 400),
           (400, 528), (528, 656), (656, 784), (784, 800)]


def _build_program(T):
    import concourse.bacc as bacc
    import concourse.mybir as mybir
    import concourse.bass as bass
    import concourse.tile as tile
    from concourse.bass import ts, ds

    dt = mybir.dt
    TB = T * BL

    nc = bacc.Bacc("TRN2", target_bir_lowering=False, debug=False, num_devices=8)

    # ---------------- I/O ----------------
    XTA = nc.dram_tensor("XTA", [402, TB], dt.float32r, kind="ExternalInput")
    W0S = nc.dram_tensor("W0S", [402, G], dt.float32r, kind="ExternalInput")
    WH0 = nc.dram_tensor("WH0", [H, G], dt.bfloat16, kind="ExternalInput")
    W1S = nc.dram_tensor("W1S", [802, G], dt.bfloat16, kind="ExternalInput")
    WH1 = nc.dram_tensor("WH1", [H, G], dt.bfloat16, kind="ExternalInput")
    MO = nc.dram_tensor("MO", [2, TB], dt.bfloat16, kind="ExternalInput")
    I32 = nc.dram_tensor("I32", [BL, BL], dt.float32, kind="ExternalInput")
    SEL0 = nc.dram_tensor("SEL0", [128, 1], dt.float32, kind="ExternalInput")
    SEL1 = nc.dram_tensor("SEL1", [128, 1], dt.float32, kind="ExternalInput")
    # transposed output layout: OUT[u, chunk, t*BL+b] = h[t, b, 128*chunk+u]
    OUT = nc.dram_tensor("OUT", [128, 4, TB], dt.float32, kind="ExternalOutput")

    with tile.TileContext(nc) as tc:
        with (
            tc.tile_pool(name="dram", bufs=1, space="DRAM") as dp,
            tc.tile_pool(name="wres", bufs=1) as wres,     # resident weight streams
            tc.tile_pool(name="stat", bufs=3) as statp,    # GEMM stationaries
            tc.tile_pool(name="psum", bufs=1, space="PSUM") as psp,
            tc.tile_pool(name="work", bufs=2) as wk,
            tc.tile_pool(name="hts", bufs=1) as htp,
            tc.tile_pool(name="cst", bufs=1) as cst,
        ):
            Sigmoid = mybir.ActivationFunctionType.Sigmoid
            Tanh = mybir.ActivationFunctionType.Tanh

            # l0 features, chunk-packed [feat-within-chunk, chunk, time*batch]
            L0T = dp.tile([128, 4, TB], dt.bfloat16, tag="L0T", name="L0T")
            EXIN = dp.tile([128, 4, TB], dt.bfloat16, tag="EXIN", name="EXIN")
            EXO = dp.tile([2, 128, 4, TB], dt.bfloat16, tag="EXO", name="EXO")
            PEER = dp.tile([128, 4, TB], dt.bfloat16, tag="PEER", name="PEER")
            NCH = max(1, TB // 2048)  # blend col chunks
            CW = TB // NCH

            # ---- residents ----
            i32 = cst.tile([BL, BL], dt.float32, tag="i32")
            nc.sync.dma_start(out=i32[:], in_=I32[:])
            sel0 = cst.tile([128, 1], dt.float32, tag="sel0")
            nc.sync.dma_start(out=sel0[:], in_=SEL0[:])
            sel1 = cst.tile([128, 1], dt.float32, tag="sel1")
            nc.sync.dma_start(out=sel1[:], in_=SEL1[:])

            wh0c = []
            for ci, (r0, r1) in enumerate(HCH):
                w = wres.tile([r1 - r0, G], dt.bfloat16, tag=f"wh0{ci}")
                nc.sync.dma_start(out=w[:], in_=WH0[r0:r1, :])
                wh0c.append(w)

            w0c = []
            for ci, (r0, r1) in enumerate(A_ROWS):
                w = wres.tile([r1 - r0, G], dt.float32r, tag=f"w0c{ci}")
                nc.sync.dma_start(out=w[:], in_=W0S[r0:r1, :])
                w0c.append(w)

            wh1c = []
            for ci, (r0, r1) in enumerate(HCH):
                w = wres.tile([r1 - r0, G], dt.bfloat16, tag=f"wh1{ci}")
                nc.sync.dma_start(out=w[:], in_=WH1[r0:r1, :])
                wh1c.append(w)

            w1c = []
            for ci, (r0, r1) in enumerate(W1_ROWS):
                rows = r1 - r0 + (2 if ci == 3 else 0)
                w = wres.tile([rows, G], dt.bfloat16, tag=f"w1c{ci}")
                nc.sync.dma_start(out=w[:r1 - r0, :], in_=W1S[r0:r1, :])
                if ci == 3:
                    nc.sync.dma_start(out=w[16:18, :], in_=W1S[800:802, :])
                w1c.append(w)

            # ---- persistent psum tiles: 3+3+1 banks ----
            acc1 = [psp.tile([128, 1536], dt.float32, tag=f"acc1{k}", name=f"acc1{k}")
                    for k in range(2)]
            # psum start=True marks the WHOLE 2KB bank pending-zero, so the
            # accumulating g1-tail pair lives in its own bank (never started:
            # memset + start=False accumulate), and the per-step transpose
            # scratch (start=True every step) in another.
            pssa = psp.tile([128, 512], dt.float32, tag="pssa", name="pssa")
            psst = psp.tile([128, 512], dt.float32, tag="psst", name="psst")
            # pssa cols: 0:64 = acc2[0], 64:128 = acc2[1]
            # psst cols: 0:128 = c-transpose chunks, 128:256 = sigma_o chunks

            def tp(out_ap, in_ap):
                nc.tensor.matmul(out_ap, in_ap, i32[:], is_transpose=True,
                                 skip_group_check=True)

            # =========== fills: deposit xg for 4 steps into one acc pair ===========
            # load_stat() issues the stationary DMAs; chunk(ci) emits one
            # contraction chunk's matmuls (spread across inter-step slots).
            def fillA_parts(k, colstart):
                statc = []

                def load_stat():
                    nc.vector.memset(pssa[:, 64 * k:64 * k + 64], 0.0)
                    for ci, (r0, r1) in enumerate(A_ROWS):
                        st = statp.tile([r1 - r0, 128], dt.float32r, tag=f"sa{k}{ci}")
                        nc.sync.dma_start(out=st[:], in_=XTA[r0:r1, ds(colstart, 128)])
                        statc.append(st)

                def chunk(ci):
                    st = statc[ci]
                    first = ci == 0
                    for (c0, cw) in NB1:
                        nc.tensor.matmul(acc1[k][:, c0:c0 + cw], st[:], w0c[ci][:, c0:c0 + cw],
                                         start=first, stop=False, skip_group_check=True)
                    nc.tensor.matmul(pssa[:, 64 * k:64 * k + 64], st[:], w0c[ci][:, 1536:1600],
                                     start=False, stop=False, skip_group_check=True)

                return load_stat, [lambda ci=ci: chunk(ci) for ci in range(4)]

            def fillC_parts(k, colstart):
                statc = []

                def load_stat():
                    nc.vector.memset(pssa[:, 64 * k:64 * k + 64], 0.0)
                    for ci, (r0, r1) in enumerate(HCH):
                        rows = r1 - r0 + (2 if ci == 3 else 0)
                        st = statp.tile([rows, 128], dt.bfloat16, tag=f"sc{k}{ci}")
                        nc.sync.dma_start(out=st[:r1 - r0, :], in_=L0T[:r1 - r0, ci, ds(colstart, 128)])
                        if ci == 3:
                            nc.sync.dma_start(out=st[16:18, :], in_=MO[:, ds(colstart, 128)])
                        statc.append((st, rows))
                    for ci, (r0, r1) in enumerate(HCH):
                        rows = r1 - r0
                        st = statp.tile([rows, 128], dt.bfloat16, tag=f"sp{k}{ci}")
                        if NO_DS:
                            nc.sync.dma_start(out=st[:], in_=L0T[:rows, ci, ds(colstart, 128)])
                        else:
                            nc.sync.dma_start(out=st[:], in_=PEER[:rows, ci, ds(colstart, 128)])
                        statc.append((st, rows))

                def chunk2(s):
                    for idx in (2 * s, 2 * s + 1):
                        st, rows = statc[idx]
                        first = idx == 0
                        w = w1c[idx]
                        for (c0, cw) in NB1:
                            nc.tensor.matmul(acc1[k][:, c0:c0 + cw], st[:rows, :], w[:rows, c0:c0 + cw],
                                             start=first, stop=False, skip_group_check=True)
                        nc.tensor.matmul(pssa[:, 64 * k:64 * k + 64], st[:rows, :], w[:rows, 1536:1600],
                                         start=False, stop=False, skip_group_check=True)

                return load_stat, [lambda s=s: chunk2(s) for s in range(4)]

            # =========== one recurrence step ===========
            # gate cols in acc pair (at rows 32u:32u+32):
            #   if0: acc1[0:512]  o0: acc1[512:768]  g0: acc1[768:1024]
            #   ifo1: acc1[1024:1456]  g1a: acc1[1456:1536]  g1b: pssa
            # Emission order = dependency order for the Tile scheduler, so each
            # consumer is emitted right after its minimal producer set while
            # the PE stream stays in execution order (mms, then transposes).
            def step_body(k, u, t, rt, whc, write_l0, write_out, hT, c, fill_slot):
                a1 = acc1[k]
                r0 = 32 * u
                rs = slice(r0, r0 + 32)
                a2 = pssa[:, 64 * k:64 * k + 64]
                a2r = pssa[rs, 64 * k:64 * k + 64]
                tpos = (0, r0)

                def mmblk(c0, cw, cis=(0, 1, 2, 3)):
                    for ci in cis:
                        hr0, hr1 = HCH[ci]
                        rows = hr1 - hr0
                        nc.tensor.matmul(a1[rs, c0:c0 + cw], hT[:rows, ci, :],
                                         whc[ci][:, c0:c0 + cw],
                                         start=False, stop=(ci == 3), skip_group_check=True,
                                         tile_position=tpos)

                sig = wk.tile([BL, 1200], dt.float32, tag="sig")
                gt = wk.tile([BL, 400], dt.float32, tag="gt")
                t1t = wk.tile([BL, 256], dt.float32, tag="t1t")
                t2t = wk.tile([BL, 256], dt.float32, tag="t2t")
                tanhT = wk.tile([128, 4, BL], dt.float32, tag="tanhT")

                mmblk(0, 512)
                nc.scalar.activation(sig[:, 0:512], a1[rs, 0:512], Sigmoid)
                nc.gpsimd.tensor_mul(t1t[:, 0:256], sig[:, 256:512], c[:, 0:256])
                mmblk(512, 512)
                nc.scalar.activation(gt[:, 0:256], a1[rs, 768:1024], Tanh)
                nc.scalar.activation(sig[:, 512:768], a1[rs, 512:768], Sigmoid)
                nc.vector.tensor_mul(t2t[:, 0:256], sig[:, 0:256], gt[:, 0:256])
                nc.vector.tensor_add(c[:, 0:256], t1t[:, 0:256], t2t[:, 0:256])
                mmblk(1024, 512)
                nc.scalar.activation(sig[:, 768:1056], a1[rs, 1024:1312], Sigmoid)
                nc.scalar.activation(gt[:, 256:400], a1[rs, 1312:1456], Tanh)
                nc.gpsimd.tensor_mul(t1t[:, 0:144], sig[:, 912:1056], c[:, 256:400])
                nc.vector.tensor_mul(t2t[:, 0:144], sig[:, 768:912], gt[:, 256:400])
                nc.vector.tensor_add(c[:, 256:400], t1t[:, 0:144], t2t[:, 0:144])
                nc.scalar.activation(sig[:, 1056:1136], a1[rs, 1456:1536], Sigmoid)
                for ci, (hr0, hr1) in enumerate(HCH):
                    rows = hr1 - hr0
                    nc.tensor.matmul(a2r, hT[:rows, ci, :], whc[ci][:, 1536:1600],
                                     start=False, stop=(ci == 3), skip_group_check=True,
                                     tile_position=tpos)
                nc.scalar.activation(sig[:, 1136:1200], a2r, Sigmoid)

                # transposes after all mms in the PE stream
                tp(psst[:, 128:160], sig[:, 512:640])
                tp(psst[:, 160:192], sig[:, 640:768])
                tp(psst[:, 0:32], c[:, 0:128])
                tp(psst[:, 32:64], c[:, 128:256])
                nc.scalar.activation(tanhT[:, 0:2, :], psst[:, 0:64], Tanh)
                nc.vector.tensor_mul(hT[:, 0:2, :], psst[:, 128:192], tanhT[:, 0:2, :])
                tp(psst[:, 64:96], c[:, 256:384])
                tp(psst[:16, 96:128], c[:, 384:400])
                nc.scalar.activation(tanhT[:, 2:4, :], psst[:, 64:128], Tanh)
                tp(psst[:, 192:224], sig[:, 1056:1184])
                tp(psst[:16, 224:256], sig[:, 1184:1200])
                nc.vector.tensor_mul(hT[:, 2:4, :], psst[:, 192:256], tanhT[:, 2:4, :])

                if write_l0:
                    nc.gpsimd.dma_start(out=L0T[:, :, ts(t, BL)], in_=hT[:])
                    nc.gpsimd.dma_start(out=EXIN[:, :, ts(rt, BL)], in_=hT[:])
                if write_out:
                    hTf = wk.tile([128, 4, BL], dt.float32, tag="hTf")
                    nc.vector.tensor_mul(hTf[:], psst[:, 128:256], tanhT[:])
                    nc.gpsimd.dma_start(out=OUT[:, :, ts(t, BL)], in_=hTf[:])

                if fill_slot is not None:
                    fill_slot()

            def exchange():
                nc.gpsimd.collective_compute(
                    "AllGather", mybir.AluOpType.bypass,
                    replica_groups=[[0, 1], [2, 3], [4, 5], [6, 7]],
                    ins=[EXIN[:]], outs=[EXO[:]],
                )
                with tc.For_i(0, NCH) as q:
                    for mci, (mr0, mr1) in enumerate(HCH):
                        e0 = wk.tile([128, CW], dt.bfloat16, tag="exm0", name="e0")
                        e1 = wk.tile([128, CW], dt.bfloat16, tag="exm1", name="e1")
                        pm = wk.tile([128, CW], dt.bfloat16, tag="exmp", name="pm")
                        rows = mr1 - mr0
                        nc.sync.dma_start(out=e0[:rows, :], in_=EXO[0, :rows, mci, ts(q, CW)])
                        nc.sync.dma_start(out=e1[:rows, :], in_=EXO[1, :rows, mci, ts(q, CW)])
                        nc.vector.tensor_scalar_mul(pm[:rows, :], e1[:rows, :], sel1[:rows, :])
                        nc.vector.scalar_tensor_tensor(
                            pm[:rows, :], e0[:rows, :], sel0[:rows, :], pm[:rows, :],
                            mybir.AluOpType.mult, mybir.AluOpType.add)
                        nc.sync.dma_start(out=PEER[:rows, mci, ts(q, CW)], in_=pm[:rows, :])

            # =========== a full recurrence (one layer-direction) ===========
            def recurrence(phase, whc, fill_parts, write_l0, write_out):
                c = cst.tile([BL, 512], dt.float32, tag=f"c{phase}")
                nc.vector.memset(c[:], 0.0)
                hT = htp.tile([128, 4, BL], dt.bfloat16, tag=f"hT{phase}")
                nc.vector.memset(hT[:], 0.0)
                # zero the pad rows of the transpose psum chunks once; the
                # per-step transposes only rewrite rows 0:16 of chunk 3
                nc.vector.memset(psst[:, 96:128], 0.0)
                nc.vector.memset(psst[:, 224:256], 0.0)

                # prologue: acc1[0] gets steps 0..3
                loadA, chunksA = fill_parts(0, 0)
                loadA()
                for ch in chunksA:
                    ch()

                def body(j, with_fa):
                    # slots 0-3 carry fB (acc1[1] for THIS body's steps 4-7),
                    # slots 4-7 carry fA (acc1[0] for the NEXT body's steps 0-3)
                    loadB, chunksB = fill_parts(1, j * 32 + 128)
                    loadB()
                    if with_fa:
                        loadA, chunksA = fill_parts(0, j * 32 + 256)
                    for u in range(4):
                        step_body(0, u, j + u, (T - 1 - u) - j, whc,
                                  write_l0, write_out, hT, c, chunksB[u])
                    if with_fa:
                        loadA()
                    for u in range(4, 8):
                        step_body(1, u - 4, j + u, (T - 1 - u) - j, whc,
                                  write_l0, write_out, hT, c,
                                  chunksA[u - 4] if with_fa else None)

                with tc.For_i(0, T - 8, 8) as j:
                    body(j, True)
                body(T - 8, False)

            recurrence(0, wh0c, fillA_parts, write_l0=True, write_out=False)
            if not NO_CC:
                exchange()
            recurrence(1, wh1c, fillC_parts, write_l0=False, write_out=True)

    nc.compile()
    return nc


_PROG_CACHE = {}


def _get_program(T):
    if T not in _PROG_CACHE:
        _PROG_CACHE[T] = _build_program(T)
    return _PROG_CACHE[T]


def _prep_core_inputs(x, lengths, wdict, T):
    """Build per-core input maps. x: [T,B,400] f32, lengths: [B] int."""
    B = x.shape[1]
    mask = (np.arange(T)[:, None] < np.asarray(lengths)[None, :])  # [T,B]
    in_maps = []
    i32 = np.eye(BL, dtype=np.float32)
    for core in range(8):
        p, d = core // 2, core % 2
        bs = slice(BL * p, BL * (p + 1))
        xl = np.asarray(x[:, bs, :], np.float32)
        ml = mask[:, bs].astype(np.float32)
        if d:
            xl, ml = xl[::-1], ml[::-1]
        TB = T * BL
        xt = np.ascontiguousarray(xl.reshape(TB, 400).T)  # [400, TB]
        negrow = (NEG * (1.0 - ml)).reshape(1, TB)
        ones = np.ones((1, TB), np.float32)
        XTA = np.concatenate([xt, negrow, ones], 0).astype(np.float32)
        MOv = np.concatenate([negrow, ones], 0).astype(BF16)

        dd = "f" if d == 0 else "b"
        wi0, whh0 = wdict[f"w_ih_{dd}0"], wdict[f"w_hh_{dd}0"]
        bi0 = wdict[f"b_ih_{dd}0"] + wdict[f"b_hh_{dd}0"]
        wi1, whh1 = wdict[f"w_ih_{dd}1"], wdict[f"w_hh_{dd}1"]
        bi1 = wdict[f"b_ih_{dd}1"] + wdict[f"b_hh_{dd}1"]

        def stream0(wi, bi):
            # [402, 1600]: rows 0:400 = wi.T permuted cols; 400 = -1 on sigma; 401 = bias
            out = np.zeros((402, G), np.float32)
            out[:400, :] = wi[PERM, :].T
            out[400, :] = np.where(SIGMA, -1.0, 0.0)
            out[401, :] = bi[PERM]
            return out.astype(np.float32)

        def stream1(wi, bi):
            # rows 0:400 contract with OWN-chain l0out features, rows
            # 400:800 with the PEER chain's. wi's columns are [f(400), b(400)].
            out = np.zeros((802, G), np.float32)
            own = wi[:, 400 * d:400 * d + 400]
            peer = wi[:, 400 * (1 - d):400 * (1 - d) + 400]
            out[:400, :] = own[PERM, :].T
            out[400:800, :] = peer[PERM, :].T
            out[800, :] = np.where(SIGMA, -1.0, 0.0)
            out[801, :] = bi[PERM]
            return out.astype(BF16)

        in_maps.append({
            "XTA": XTA,
            "W0S": stream0(wi0, bi0),
            "WH0": np.ascontiguousarray(whh0[PERM, :].T).astype(BF16),
            "W1S": stream1(wi1, bi1),
            "WH1": np.ascontiguousarray(whh1[PERM, :].T).astype(BF16),
            "MO": MOv,
            "I32": i32,
            "SEL0": np.full((128, 1), float(d), np.float32),
            "SEL1": np.full((128, 1), 1.0 - float(d), np.float32),
        })
    return in_maps


def _unpack_out(res_out, T):
    # OUT[u, chunk, t*BL+b] -> h[t, b, 128*chunk+u], valid units 0:400
    o = np.asarray(res_out).reshape(128, 4, T, BL)
    return o.transpose(2, 3, 1, 0).reshape(T, BL, 512)[:, :, :400]


def _run(x, lengths, wdict, T):
    from concourse.bass_utils import run_bass_kernel_spmd

    nc = _get_program(T)
    in_maps = _prep_core_inputs(x, lengths, wdict, T)
    res = run_bass_kernel_spmd(nc, in_maps, list(range(8)))
    B = x.shape[1]
    out = np.zeros((T, B, 2 * H), np.float32)
    for core in range(8):
        p, d = core // 2, core % 2
        hl = _unpack_out(res.results[core]["OUT"], T)
        if d:
            hl = hl[::-1]
        out[:, BL * p:BL * (p + 1), H * d:H * (d + 1)] = hl
    return out


def kernel(x, lengths, **weights):
    x = np.asarray(x, np.float32)
    lengths = np.asarray(lengths)
    wd = {k: np.asarray(v, np.float32) for k, v in weights.items()}
    return _run(x, lengths, wd, x.shape[0])
